# revision 9
# baseline (speedup 1.0000x reference)
"""Trainium2 Bass kernel for nn_DiscreteAutoregressiveFlow (sampling, forward).

Math: `inputs` is an exact one-hot [B, L, V] tensor. For a row holding token v:
  net = W[v] + b                      (exact: one-hot @ W picks a row)
  loc = one_hot(argmax(net[:V]));  scale = one_hot(argmax(net[V:]))
  one_hot_multiply -> one-hot at (scale_tok*v) % V   (zero row if scale_tok==0)
  one_hot_add      -> one-hot at (scale_tok*v + loc_tok) % V
So out[row] = one_hot(cmap[v]) with a host-precomputed 64-entry map
(sentinel >= V encodes the zero row). The straight-through softmax residuals
and FFT noise in the reference are O(1e-7) and vanish in norm relative error.

Device pipeline per 128x(R*64) chunk (pure streaming, memory-bound):
  prod(slot) = 1 + cmap/128 tiled     (scalar engine broadcast fill, startup)
  prod += x                           (DMA-in with inline add, SWDGE CCE)
  m   = reduce_max(prod, inner V)     (DVE) = 1 + cmap[tok]/128, exact
  out = is_equal(1 + iota/128, m)     (DVE) -> one-hot rows, exact 0.0/1.0
All f32 values involved are exact (c <= 127 and 2^-7 scaling), so the
comparison is exact. Every instruction carries at most one semaphore wait
(the TT/reduce ISA structs only have one wait slot); buffers are fully
unrolled per chunk so no WAR waits exist.
Sharding: pure data parallel over B*L rows, 8 cores, no collectives.
"""

import numpy as np

V = 64
P = 128
N_CORES = 8
B, L = 16, 8192
ROWS = B * L                      # 131072
ROWS_PER_CORE = ROWS // N_CORES   # 16384
SENTINEL = 100.0
EPS = 1.0 / 128.0

# rows per partition per chunk; chunk = [128, R*64] f32 = R*32KB
R = 16

_CACHE = {}


def _build_nc(rows_per_core: int, r: int):
    import concourse.bacc as bacc
    import concourse.bass as bass
    import concourse.mybir as mybir
    from concourse.bass import broadcast_tensor_aps
    from concourse.tile import TileContext

    f32 = mybir.dt.float32
    fd = r * V
    chunk_rows = P * r
    n_chunks = rows_per_core // chunk_rows
    assert rows_per_core % chunk_rows == 0

    # Bacc (not raw Bass): its compile() runs generate_event_semaphores(),
    # which legalizes multi-wait instructions for TRN2 (1 wait per instr).
    nc = bacc.Bacc("TRN2", target_bir_lowering=False, name="daf_onehot")
    x = nc.dram_tensor("x", [rows_per_core, V], f32, kind="ExternalInput")
    cmap = nc.dram_tensor("cmap", [P, V], f32, kind="ExternalInput")
    iota = nc.dram_tensor("iota", [P, V], f32, kind="ExternalInput")
    y = nc.dram_tensor("y", [rows_per_core, V], f32, kind="ExternalOutput")

    xv = x.rearrange("(c p r) v -> c p (r v)", p=P, r=r)
    yv = y.rearrange("(c p r) v -> c p (r v)", p=P, r=r)

    with TileContext(nc) as tc:
        with (
            tc.tile_pool(name="const", bufs=1) as constp,
            tc.tile_pool(name="io", bufs=n_chunks) as iop,
            tc.tile_pool(name="work", bufs=n_chunks) as workp,
        ):
            cmap_st = constp.tile([P, V], f32, tag="cmap_st")
            iota_st = constp.tile([P, V], f32, tag="iota_st")
            nc.sync.dma_start(cmap_st[:], cmap[:])
            nc.sync.dma_start(iota_st[:], iota[:])
            # eq reads iota through a vector-owned tile so the eq's deps
            # collapse onto the vector self-semaphore (single wait slot).
            iota_v = constp.tile([P, V], f32, tag="iota_v")
            nc.vector.tensor_copy(iota_v[:], iota_st[:])
            cmap_1 = cmap_st[:].rearrange("p (one v) -> p one v", one=1)
            iota_1 = iota_v[:].rearrange("p (one v) -> p one v", one=1)

            prods = []
            for ci in range(n_chunks):
                prod = workp.tile([P, fd], f32, tag="prod")
                p3 = prod[:].rearrange("p (r v) -> p r v", v=V)
                cm_b, _ = broadcast_tensor_aps(cmap_1, p3)
                nc.scalar.copy(p3, cm_b)
                prods.append(prod)

            # Domination trick: scalar writes a marker after all fills (ACT is
            # in-order), vector copies it once. The vector clock then covers
            # every fill, so the per-chunk reduce only waits on its accum-DMA
            # (the reduce ISA struct has a single wait slot).
            marker = constp.tile([P, 1], f32, tag="marker")
            nc.scalar.copy(marker[:], cmap_st[:, 0:1])
            marker_v = constp.tile([P, 1], f32, tag="marker_v")
            nc.vector.tensor_copy(marker_v[:], marker[:])

            for ci in range(n_chunks):
                prod = prods[ci]
                p3 = prod[:].rearrange("p (r v) -> p r v", v=V)
                nc.gpsimd.dma_start(
                    prod[:], xv[ci], accum_op=mybir.AluOpType.add
                )

                c_t = workp.tile([P, r], f32, tag="c")
                nc.vector.tensor_reduce(
                    c_t[:], p3, axis=mybir.AxisListType.X, op=mybir.AluOpType.max
                )

                out_t = iop.tile([P, fd], f32, tag="out")
                o3 = out_t[:].rearrange("p (r v) -> p r v", v=V)
                io_b, _ = broadcast_tensor_aps(iota_1, o3)
                c3 = c_t[:].rearrange("p (r one) -> p r one", one=1)
                c3_b, _ = broadcast_tensor_aps(c3, o3)
                nc.vector.tensor_tensor(o3, io_b, c3_b, op=mybir.AluOpType.is_equal)

                nc.sync.dma_start(yv[ci], out_t[:])

    # Bacc.finalize runs compile(): wait-splitting (generate_event_semaphores),
    # register allocation, nop fusion. run_bass_via_pjrt serializes nc.m as-is,
    # so this must happen here.
    nc.finalize()
    return nc


def _get_nc(rows_per_core=ROWS_PER_CORE, r=R):
    key = (rows_per_core, r)
    if key not in _CACHE:
        _CACHE[key] = _build_nc(rows_per_core, r)
    return _CACHE[key]


def _host_cmap(W: np.ndarray, b: np.ndarray) -> np.ndarray:
    """64-entry map token -> output one-hot index (or sentinel for zero row)."""
    net = W.astype(np.float32) + b.astype(np.float32)[None, :]   # [V, 2V]
    loc_tok = np.argmax(net[:, :V], axis=1)                      # [V]
    scale_tok = np.argmax(net[:, V:], axis=1)                    # [V]
    t = (scale_tok * np.arange(V, dtype=np.int64) + loc_tok) % V
    return np.where(scale_tok == 0, SENTINEL, t.astype(np.float64)).astype(
        np.float32
    )


def _host_tables(W: np.ndarray, b: np.ndarray):
    cmap_eps = _host_cmap(W, b) * np.float32(EPS)                  # exact f32
    iota_eps = 1.0 + np.arange(V, dtype=np.float32) * np.float32(EPS)
    cmap_t = np.tile(cmap_eps.astype(np.float32)[None, :], (P, 1))
    iota_t = np.tile(iota_eps.astype(np.float32)[None, :], (P, 1))
    return cmap_t, iota_t


def kernel(inputs: np.ndarray, W: np.ndarray, b: np.ndarray) -> np.ndarray:
    from concourse import bass_utils

    x = np.ascontiguousarray(inputs.astype(np.float32, copy=False).reshape(ROWS, V))
    cmap_t, iota_t = _host_tables(W, b)

    nc = _get_nc()
    in_maps = [
        {
            "x": x[c * ROWS_PER_CORE : (c + 1) * ROWS_PER_CORE],
            "cmap": cmap_t,
            "iota": iota_t,
        }
        for c in range(N_CORES)
    ]
    res = bass_utils.run_bass_kernel_spmd(nc, in_maps, core_ids=list(range(N_CORES)))
    y = np.concatenate([r["y"] for r in res.results], axis=0)
    return y.reshape(inputs.shape).astype(inputs.dtype, copy=False)


# revision 23
# speedup vs baseline: 1.1630x; 1.1630x over previous
"""Trainium2 Bass kernel for nn_DiscreteAutoregressiveFlow (sampling, forward).

Math: `inputs` is an exact one-hot [B, L, V] tensor. For a row holding token v:
  net = W[v] + b                      (exact: one-hot @ W picks a row)
  loc = one_hot(argmax(net[:V]));  scale = one_hot(argmax(net[V:]))
  one_hot_multiply -> one-hot at (scale_tok*v) % V   (zero row if scale_tok==0)
  one_hot_add      -> one-hot at (scale_tok*v + loc_tok) % V
So out[row] = one_hot(cmap[v]) with a host-precomputed 64-entry map
(sentinel >= V encodes the zero row). The straight-through softmax residuals
and FFT noise in the reference are O(1e-7) and vanish in norm relative error.

Device pipeline per 128x(R*64) chunk (pure streaming, memory-bound):
  xt   = DMA-in (HWDGE, plain)
  prod = xt + cmap/128                (gpsimd tensor_tensor add)
  m    = reduce_max(prod, inner V)    (DVE) = 1 + cmap[tok]/128, exact
  out  = is_equal(1 + iota/128, m)    (DVE) -> one-hot rows, exact 0.0/1.0
  DMA-out (HWDGE)
All f32 values involved are exact (c <= 127 and 2^-7 scaling), so the
comparison is exact. Buffers are fully unrolled per chunk (no WAR waits);
excess waits are legalized by Bacc's generate_event_semaphores.
Sharding: pure data parallel over B*L rows, 8 cores, no collectives.
"""

import numpy as np

V = 64
P = 128
N_CORES = 8
B, L = 16, 8192
ROWS = B * L                      # 131072
ROWS_PER_CORE = ROWS // N_CORES   # 16384
SENTINEL = 100.0
EPS = 1.0 / 128.0

# rows per partition per chunk; chunk = [128, R*64] f32 = R*32KB
R = 16
# Of the 2*n_chunks add/eq ops, how many run on gpsimd (the rest on DVE).
# gpsimd TT is ~2.35x slower per element than DVE TT; ~7/8 adds on gpsimd
# balances the engines (reduce and is_equal are DVE-only at the ISA level).
N_GPSIMD_ADD = 0
N_GPSIMD_EQ = 0

_CACHE = {}


def _build_nc(rows_per_core: int, r: int, n_gp_add: int = N_GPSIMD_ADD,
              n_gp_eq: int = N_GPSIMD_EQ, row_major_partitions: bool = False):
    import concourse.bacc as bacc
    import concourse.bass as bass
    import concourse.mybir as mybir
    from concourse.bass import broadcast_tensor_aps
    from concourse.tile import TileContext

    f32 = mybir.dt.float32
    fd = r * V
    chunk_rows = P * r
    n_chunks = rows_per_core // chunk_rows
    assert rows_per_core % chunk_rows == 0

    # Bacc (not raw Bass): its compile() runs generate_event_semaphores(),
    # which legalizes multi-wait instructions for TRN2 (1 wait per instr).
    nc = bacc.Bacc("TRN2", target_bir_lowering=False, name="daf_onehot")
    x = nc.dram_tensor("x", [rows_per_core, V], f32, kind="ExternalInput")
    cmap = nc.dram_tensor("cmap", [P, V], f32, kind="ExternalInput")
    iota = nc.dram_tensor("iota", [P, V], f32, kind="ExternalInput")
    y = nc.dram_tensor("y", [rows_per_core, V], f32, kind="ExternalOutput")

    if row_major_partitions:
        xv = x.rearrange("(c r p) v -> c p r v", p=P, r=r)
        yv = y.rearrange("(c r p) v -> c p r v", p=P, r=r)
    else:
        xv = x.rearrange("(c p r) v -> c p (r v)", p=P, r=r)
        yv = y.rearrange("(c p r) v -> c p (r v)", p=P, r=r)

    with TileContext(nc) as tc:
        with (
            tc.tile_pool(name="const", bufs=1) as constp,
            tc.tile_pool(name="io", bufs=n_chunks) as iop,
            tc.tile_pool(name="work", bufs=n_chunks) as workp,
        ):
            cmap_st = constp.tile([P, V], f32, tag="cmap_st")
            iota_st = constp.tile([P, V], f32, tag="iota_st")
            nc.sync.dma_start(cmap_st[:], cmap[:])
            nc.sync.dma_start(iota_st[:], iota[:])
            # Each engine reads the constants through its own copy so the
            # hot-loop deps collapse onto that engine's self-semaphore.
            iota_v = constp.tile([P, V], f32, tag="iota_v")
            nc.vector.tensor_copy(iota_v[:], iota_st[:])
            cmap_v = constp.tile([P, V], f32, tag="cmap_v")
            nc.vector.tensor_copy(cmap_v[:], cmap_st[:])
            iota_g = constp.tile([P, V], f32, tag="iota_g")
            nc.gpsimd.tensor_copy(iota_g[:], iota_st[:])
            cmap_g = constp.tile([P, V], f32, tag="cmap_g")
            nc.gpsimd.tensor_copy(cmap_g[:], cmap_st[:])
            one_of = {
                (nc.vector, "cmap"): cmap_v[:].rearrange("p (o v) -> p o v", o=1),
                (nc.vector, "iota"): iota_v[:].rearrange("p (o v) -> p o v", o=1),
                (nc.gpsimd, "cmap"): cmap_g[:].rearrange("p (o v) -> p o v", o=1),
                (nc.gpsimd, "iota"): iota_g[:].rearrange("p (o v) -> p o v", o=1),
            }

            for ci in range(n_chunks):
                # Spread the gpsimd-assigned ops across chunk indices so the
                # two engines interleave rather than serialize phase-wise.
                add_eng = nc.gpsimd if (ci * n_gp_add) % n_chunks < n_gp_add else nc.vector
                eq_eng = nc.gpsimd if (ci * n_gp_eq) % n_chunks < n_gp_eq else nc.vector

                xt = iop.tile([P, fd], f32, tag="x")
                x3 = xt[:].rearrange("p (r v) -> p r v", v=V)
                nc.sync.dma_start(x3 if row_major_partitions else xt[:], xv[ci])

                prod = workp.tile([P, fd], f32, tag="prod")
                p3 = prod[:].rearrange("p (r v) -> p r v", v=V)
                cm_b, _ = broadcast_tensor_aps(one_of[(add_eng, "cmap")], x3)
                add_eng.tensor_tensor(p3, x3, cm_b, op=mybir.AluOpType.add)

                c_t = workp.tile([P, r], f32, tag="c")
                nc.vector.tensor_reduce(
                    c_t[:], p3, axis=mybir.AxisListType.X, op=mybir.AluOpType.max
                )

                out_t = iop.tile([P, fd], f32, tag="out")
                o3 = out_t[:].rearrange("p (r v) -> p r v", v=V)
                io_b, _ = broadcast_tensor_aps(one_of[(eq_eng, "iota")], o3)
                c3 = c_t[:].rearrange("p (r one) -> p r one", one=1)
                c3_b, _ = broadcast_tensor_aps(c3, o3)
                eq_eng.tensor_tensor(o3, io_b, c3_b, op=mybir.AluOpType.is_equal)

                nc.sync.dma_start(yv[ci], o3 if row_major_partitions else out_t[:])

    # Bacc.finalize runs compile(): wait-splitting (generate_event_semaphores),
    # register allocation, nop fusion. run_bass_via_pjrt serializes nc.m as-is,
    # so this must happen here.
    nc.finalize()
    return nc


def _get_nc(rows_per_core=ROWS_PER_CORE, r=R, n_gp_add=N_GPSIMD_ADD,
            n_gp_eq=N_GPSIMD_EQ, row_major_partitions=False):
    key = (rows_per_core, r, n_gp_add, n_gp_eq, row_major_partitions)
    if key not in _CACHE:
        _CACHE[key] = _build_nc(rows_per_core, r, n_gp_add, n_gp_eq,
                                row_major_partitions)
    return _CACHE[key]


def _host_cmap(W: np.ndarray, b: np.ndarray) -> np.ndarray:
    """64-entry map token -> output one-hot index (or sentinel for zero row)."""
    net = W.astype(np.float32) + b.astype(np.float32)[None, :]   # [V, 2V]
    loc_tok = np.argmax(net[:, :V], axis=1)                      # [V]
    scale_tok = np.argmax(net[:, V:], axis=1)                    # [V]
    t = (scale_tok * np.arange(V, dtype=np.int64) + loc_tok) % V
    return np.where(scale_tok == 0, SENTINEL, t.astype(np.float64)).astype(
        np.float32
    )


def _host_tables(W: np.ndarray, b: np.ndarray):
    cmap_eps = _host_cmap(W, b) * np.float32(EPS)                  # exact f32
    iota_eps = 1.0 + np.arange(V, dtype=np.float32) * np.float32(EPS)
    cmap_t = np.tile(cmap_eps.astype(np.float32)[None, :], (P, 1))
    iota_t = np.tile(iota_eps.astype(np.float32)[None, :], (P, 1))
    return cmap_t, iota_t


def kernel(inputs: np.ndarray, W: np.ndarray, b: np.ndarray) -> np.ndarray:
    from concourse import bass_utils

    x = np.ascontiguousarray(inputs.astype(np.float32, copy=False).reshape(ROWS, V))
    cmap_t, iota_t = _host_tables(W, b)

    nc = _get_nc()
    in_maps = [
        {
            "x": x[c * ROWS_PER_CORE : (c + 1) * ROWS_PER_CORE],
            "cmap": cmap_t,
            "iota": iota_t,
        }
        for c in range(N_CORES)
    ]
    res = bass_utils.run_bass_kernel_spmd(nc, in_maps, core_ids=list(range(N_CORES)))
    y = np.concatenate([r["y"] for r in res.results], axis=0)
    return y.reshape(inputs.shape).astype(inputs.dtype, copy=False)
